# revision 16
# baseline (speedup 1.0000x reference)
"""ChebNet (K=2) GNN message passing on 8 TRN2 NeuronCores.

Strategy v2 (edge sharding by destination stripe, gpsimd ap_gather):
  - Sort edges by destination node; core c owns destinations
    [c*6272, (c+1)*6272) (N padded 50000 -> 50176 = 8*6272).
  - The gather table x' = dinv (.) x is kept FEATURE-MAJOR in SBUF:
    tbl[128, 25088] fp32 where partitions 0..63 hold features 0..63 of
    nodes 0..25087 (lo half) and partitions 64..127 hold features of
    nodes 25088..50175 (hi half). Indices are int16 node-ids local to a
    half (< 25088 < 2^15).
  - Per-edge gather runs on the GPSIMD engine via InstAPGather
    (~21ns/idx, ~68us/call fixed) in NCALL big calls per matvec. Each
    call gathers CH 128-edge blocks of the lo band (partitions 0..63)
    and CH blocks of the hi band (64..127) concurrently (the two edge
    streams are band-local column streams of the idx tensor).
  - Gathered blocks [64 feat, 128 edge] are PE-transposed to
    [128 edge, 64], then scattered into the 128-dest window accumulator
    via one-hot matmuls (S = (iota == local_dest)), all bf16 except the
    fp32 PSUM accumulation. The drain scales by dinv[dest] (the minus
    sign of L_hat is folded into W1 on the host).
  - Dense layers + log_softmax are streamed per 128-node window right
    after each window's matvec drain; only xT and hT [64, 6272] bf16
    persist. Layer 2 repeats with the table reloaded from the h'
    AllGather.
"""

import os
import sys

import numpy as np

sys.path.insert(0, "/opt/trn_rl_repo")

import concourse.bacc as bacc
import concourse.bass as bass
import concourse.tile as tile
from concourse import mybir
from concourse.masks import make_identity

FP32 = mybir.dt.float32
BF16 = mybir.dt.bfloat16
I32 = mybir.dt.int32
I16 = mybir.dt.int16

N = 50000
E = 800000
F = 64          # in dim
HID = 64
OUT = 40
C = 8           # cores
SN = 6272       # nodes per stripe (49 * 128)
NP = SN * C     # padded node count 50176
W = SN // 128   # 49 windows per core
HALF = NP // 2  # 25088, int16-indexable table half
AX = mybir.AxisListType
NCALL = int(os.environ.get("CHEB_NCALL", "10"))   # gather calls per matvec


# ---------------------------------------------------------------- host side


def _pack(edge_index: np.ndarray):
    """Integer-only preprocessing: sort/partition/pad the edge list.

    Edges are bucketed by (dest window, source half) and padded to whole
    128-edge blocks; block counts per (window, half) are maxed over
    cores so the SPMD program is uniform. Two independent block streams
    result: the lo stream (sources < HALF, gathered on partitions
    0..63) and the hi stream (partitions 64..127).

    Returns:
      idxw  [C, 128, NBB*8] int16 - ap_gather indices, wrapped (s p)
            per 16-partition group, replicated x4 down each band.
      ldst  [C, 128, NBL+NBH] f32 - local dest slot (0..127) or 255;
            lo stream at cols [0, NBL), hi at [NBL, NBL+NBH).
      rp_a/rp_b [C, 128, W] f32 - CSR row_ptr (degree = rp_b - rp_a).
      groups: per window (lo_b, lo_n, hi_b, hi_n) band block offsets.
      NBL, NBH: total blocks per band.
    """
    row = np.asarray(edge_index[0], dtype=np.int64)
    col = np.asarray(edge_index[1], dtype=np.int64)

    cnt = np.bincount(row, minlength=NP)
    rp = np.zeros(NP + 1, dtype=np.int64)
    np.cumsum(cnt, out=rp[1:])

    order = np.argsort(row, kind="stable")
    rs = row[order]
    cs = col[order]
    keep = rs != cs
    rs = rs[keep]
    cs = cs[keep]

    win = rs >> 7
    half = (cs >= HALF).astype(np.int64)
    gid = win * 2 + half
    gorder = np.argsort(gid, kind="stable")
    rs = rs[gorder]
    cs = cs[gorder]
    gid = gid[gorder]

    gcnt = np.bincount(gid, minlength=C * W * 2).reshape(C, W, 2)
    nbw = (gcnt.max(axis=0) + 127) // 128          # [W, 2]
    # guarantee each window has at least one block so PSUM is initialized
    nbw[nbw[:, 0] == 0, 0] = 1

    lo_b = np.zeros(W + 1, dtype=np.int64)
    np.cumsum(nbw[:, 0], out=lo_b[1:])
    hi_b = np.zeros(W + 1, dtype=np.int64)
    np.cumsum(nbw[:, 1], out=hi_b[1:])
    NBL = int(lo_b[-1])
    NBH = int(hi_b[-1])
    NBB = max(NBL, NBH)
    groups = [
        (int(lo_b[w]), int(nbw[w, 0]), int(hi_b[w]), int(nbw[w, 1]))
        for w in range(W)
    ]

    flat_lo = np.zeros((C, NBB * 128), dtype=np.int16)
    flat_hi = np.zeros((C, NBB * 128), dtype=np.int16)
    ldst = np.full((C, 128, NBL + NBH), 255.0, dtype=np.float32)

    starts = np.zeros(C * W * 2 + 1, dtype=np.int64)
    np.cumsum(gcnt.reshape(-1), out=starts[1:])
    for c in range(C):
        for w in range(W):
            for h in range(2):
                g = (c * W + w) * 2 + h
                s, e = starts[g], starts[g + 1]
                m = e - s
                if m == 0:
                    continue
                base = lo_b[w] if h == 0 else hi_b[w]
                ei = np.arange(m)
                b = base + (ei >> 7)
                p = ei & 127
                flat = flat_lo if h == 0 else flat_hi
                flat[c, b * 128 + p] = (cs[s:e] - h * HALF).astype(np.int16)
                loff = 0 if h == 0 else NBL
                ldst[c, p, loff + b] = (rs[s:e] - (c * SN + w * 128)).astype(
                    np.float32)

    # wrap (s p) per 16-partition group, replicate x4 down each band
    idxw = np.zeros((C, 128, NBB * 8), dtype=np.int16)
    lo_w = flat_lo.reshape(C, NBB * 8, 16).transpose(0, 2, 1)  # [C,16,S]
    hi_w = flat_hi.reshape(C, NBB * 8, 16).transpose(0, 2, 1)
    for grp in range(4):
        idxw[:, grp * 16:(grp + 1) * 16, :] = lo_w
        idxw[:, 64 + grp * 16:64 + (grp + 1) * 16, :] = hi_w

    rpf = rp.astype(np.float64)
    assert rpf.max() < 2 ** 24
    idx = (np.arange(W)[None, :] * 128 + np.arange(128)[:, None])
    rp_a = np.zeros((C, 128, W), dtype=np.float32)
    rp_b = np.zeros((C, 128, W), dtype=np.float32)
    for c in range(C):
        base = c * SN
        rp_a[c] = rpf[base + idx].astype(np.float32)
        rp_b[c] = rpf[base + idx + 1].astype(np.float32)

    return idxw, ldst, rp_a, rp_b, groups, NBL, NBH


# -------------------------------------------------------------- bass program


def _build(groups, NBL, NBH, replay=0):
    nc = bacc.Bacc(
        "TRN2",
        target_bir_lowering=False,
        debug=False,
        num_devices=C,
    )

    NBB = max(NBL, NBH)
    CH = (NBB + NCALL - 1) // NCALL      # blocks per gather call
    ncall = (NBB + CH - 1) // CH
    nwmax = max(g[1] + g[3] for g in groups)

    # --- I/O
    x_st = nc.dram_tensor("x_st", [SN, F], FP32, kind="ExternalInput").ap()
    idxw_d = nc.dram_tensor("idxw", [128, NBB * 8], I16,
                            kind="ExternalInput").ap()
    ldst_d = nc.dram_tensor("ldst", [128, NBL + NBH], FP32,
                            kind="ExternalInput").ap()
    rpa = nc.dram_tensor("rpa", [128, W], FP32, kind="ExternalInput").ap()
    rpb = nc.dram_tensor("rpb", [128, W], FP32, kind="ExternalInput").ap()
    w01 = nc.dram_tensor("w01", [F, HID], FP32, kind="ExternalInput").ap()
    w11 = nc.dram_tensor("w11", [F, HID], FP32, kind="ExternalInput").ap()
    b1 = nc.dram_tensor("b1", [HID, 1], FP32, kind="ExternalInput").ap()
    w02 = nc.dram_tensor("w02", [HID, OUT], FP32, kind="ExternalInput").ap()
    w12 = nc.dram_tensor("w12", [HID, OUT], FP32, kind="ExternalInput").ap()
    b2 = nc.dram_tensor("b2", [OUT, 1], FP32, kind="ExternalInput").ap()
    out = nc.dram_tensor("out", [SN, OUT], FP32, kind="ExternalOutput").ap()

    # --- internal DRAM (collective bounce)
    xp_b = nc.dram_tensor("xp_b", [64, SN], FP32).ap()
    xp_full = nc.dram_tensor("xp_full", [8 * 64, SN], FP32,
                             addr_space="Shared").ap()
    hp_b = nc.dram_tensor("hp_b", [64, SN], FP32).ap()
    hp_full = nc.dram_tensor("hp_full", [8 * 64, SN], FP32,
                             addr_space="Shared").ap()

    rg = [list(range(C))]
    DBG = os.environ.get("CHEB_DBG", "0") == "1"
    if DBG:
        t_dbg = nc.dram_tensor("t_dbg", [64, SN], FP32,
                               kind="ExternalOutput").ap()

    with tile.TileContext(nc) as tc:
        with (
            tc.tile_pool(name="const", bufs=1) as cpool,
            tc.tile_pool(name="gab", bufs=2) as gapool,
            tc.tile_pool(name="big", bufs=1) as bpool,
            tc.tile_pool(name="work", bufs=4) as wpool,
            tc.tile_pool(name="gswork", bufs=6) as gpool,
            tc.tile_pool(name="spool", bufs=2) as spool,
            tc.tile_pool(name="psw", bufs=2, space="PSUM") as psw,
            tc.tile_pool(name="pstp", bufs=2, space="PSUM") as pstp,
            tc.tile_pool(name="psd", bufs=4, space="PSUM") as psd,
        ):
            tbl_t = cpool.tile([128, HALF], FP32, tag="tbl")
            tbl = tbl_t[:]
            idxw_t = cpool.tile([128, NBB * 8], I16, tag="idxw")
            idxw_s = idxw_t[:]

            # ---- constants
            ident = cpool.tile([128, 128], FP32, tag="ident")
            make_identity(nc, ident[:])
            iota_i = cpool.tile([128, 128], I32, tag="iota_i")
            nc.gpsimd.iota(iota_i[:], pattern=[[1, 128]], base=0,
                           channel_multiplier=0)
            iota_f = cpool.tile([128, 128], BF16, tag="iota_f")
            nc.vector.tensor_copy(iota_f[:], iota_i[:])

            w01_s = cpool.tile([F, HID], BF16, tag="w01")
            w11_s = cpool.tile([F, HID], BF16, tag="w11")
            w02_s = cpool.tile([HID, OUT], BF16, tag="w02")
            w12_s = cpool.tile([HID, OUT], BF16, tag="w12")
            b1_s = cpool.tile([HID, 1], FP32, tag="b1")
            b2_s = cpool.tile([OUT, 1], FP32, tag="b2")
            for dst, src in ((b1_s, b1), (b2_s, b2)):
                nc.sync.dma_start(out=dst[:], in_=src)
            for dst, src, shp in ((w01_s, w01, [F, HID]),
                                  (w11_s, w11, [F, HID]),
                                  (w02_s, w02, [HID, OUT]),
                                  (w12_s, w12, [HID, OUT])):
                wtmp = wpool.tile(shp, FP32, tag="wtmp")
                nc.sync.dma_start(out=wtmp[:], in_=src)
                nc.vector.tensor_copy(dst[:], wtmp[:])

            nc.sync.dma_start(out=idxw_s, in_=idxw_d)
            ldst_f = spool.tile([128, NBL + NBH], FP32, tag="S")
            nc.sync.dma_start(out=ldst_f[:], in_=ldst_d)
            ldst_s = cpool.tile([128, NBL + NBH], BF16, tag="ldst")
            nc.vector.tensor_copy(ldst_s[:], ldst_f[:])

            # ---- degree -> dinv  [128, W] (node n = w*128 + p)
            rpa_s = cpool.tile([128, W], FP32, tag="rpa")
            rpb_s = cpool.tile([128, W], FP32, tag="rpb")
            nc.sync.dma_start(out=rpa_s[:], in_=rpa)
            nc.sync.dma_start(out=rpb_s[:], in_=rpb)
            deg = cpool.tile([128, W], FP32, tag="deg")
            nc.vector.tensor_tensor(out=deg[:], in0=rpb_s[:], in1=rpa_s[:],
                                    op=mybir.AluOpType.subtract)
            degc = cpool.tile([128, W], FP32, tag="degc")
            nc.vector.tensor_scalar_max(degc[:], deg[:], 1.0)
            rec = cpool.tile([128, W], FP32, tag="rec")
            nc.vector.reciprocal(rec[:], degc[:])
            rsq = cpool.tile([128, W], FP32, tag="rsq")
            nc.scalar.sqrt(rsq[:], rec[:])
            msk = cpool.tile([128, W], FP32, tag="msk")
            nc.vector.tensor_scalar(out=msk[:], in0=deg[:], scalar1=0.5,
                                    scalar2=None, op0=mybir.AluOpType.is_ge)
            dinv = cpool.tile([128, W], FP32, tag="dinv")
            nc.vector.tensor_tensor(out=dinv[:], in0=rsq[:], in1=msk[:],
                                    op=mybir.AluOpType.mult)

            # ---- persistent transposed tensors
            xT = bpool.tile([F, SN], BF16, tag="xT")
            hT = bpool.tile([HID, SN], BF16, tag="hT")

            def dinvrep_w(w):
                """[64, 128] fp32: dinv of window w's nodes, replicated
                down partitions (per-column scale for the drain)."""
                dr_ps = psd.tile([128, 128], FP32, tag="pt")
                nc.tensor.transpose(
                    out=dr_ps[:],
                    in_=dinv[:, w:w + 1].to_broadcast([128, 128]),
                    identity=ident[:])
                dr = wpool.tile([64, 128], FP32, tag="dinvrep")
                nc.vector.tensor_copy(dr[:], dr_ps[:64, :])
                return dr

            # ---- prelude: xT (bf16) + x' table stripe -> DRAM AllGather
            for w in range(W):
                xblk = wpool.tile([128, F], FP32, tag="xblk")
                nc.sync.dma_start(
                    out=xblk[:],
                    in_=bass.AP(x_st.tensor, x_st.offset + w * 128 * F,
                                [[F, 128], [1, F]]))
                xt_ps = psd.tile([F, 128], FP32, tag="pt")
                nc.tensor.transpose(out=xt_ps[:], in_=xblk[:],
                                    identity=ident[:])
                nc.vector.tensor_copy(xT[:, w * 128:(w + 1) * 128], xt_ps[:])
                dr = dinvrep_w(w)
                xpblk = wpool.tile([64, 128], FP32, tag="pblk")
                nc.vector.tensor_tensor(out=xpblk[:], in0=xt_ps[:],
                                        in1=dr[:], op=mybir.AluOpType.mult)
                nc.sync.dma_start(
                    out=bass.AP(xp_b.tensor, xp_b.offset + w * 128,
                                [[SN, 64], [1, 128]]),
                    in_=xpblk[:])
            nc.gpsimd.collective_compute(
                "AllGather", mybir.AluOpType.bypass,
                ins=[xp_b], outs=[xp_full], replica_groups=rg)

            def load_table(full):
                # stripes 0..3 -> lo band (parts 0..63), 4..7 -> hi band
                for s in range(8):
                    pbase = 0 if s < 4 else 64
                    cbase = (s % 4) * SN
                    nc.sync.dma_start(
                        out=tbl[pbase:pbase + 64, cbase:cbase + SN],
                        in_=full[s * 64:(s + 1) * 64, :])

            load_table(xp_full)

            # ---- matvec + streamed dense layer
            def matvec_layer(layer, do_gather=True, do_compute=True):
                gatiles = {}

                def issue_call(k):
                    if k in gatiles or k >= ncall:
                        return
                    b0 = k * CH
                    nblk = min(CH, NBB - b0)
                    gat = gapool.tile([128, CH * 128], FP32, tag="ga",
                                      name="ga%d" % k)
                    ga = gat[:]
                    gatiles[k] = ga
                    if do_gather:
                        nc.gpsimd.ap_gather(
                            out_ap=ga[:, :nblk * 128],
                            in_ap=tbl,
                            idxs_ap=idxw_s[:, b0 * 8:(b0 + nblk) * 8],
                            channels=128,
                            num_elems=HALF,
                            d=1,
                            num_idxs=nblk * 128,
                        )

                issue_call(0)
                issue_call(1)
                if not do_compute:
                    # consume one element so replay variants aren't dead
                    for k in range(2, ncall):
                        issue_call(k)
                    dummy = wpool.tile([1, 1], FP32, tag="dummy")
                    nc.vector.tensor_copy(dummy[:], gatiles[0][:1, :1])
                    return
                for w in range(W):
                    lo_b, lo_n, hi_b, hi_n = groups[w]
                    ntot = lo_n + hi_n
                    # issue gather calls just-in-time, one call ahead
                    need = max(lo_b + lo_n, hi_b + hi_n) - 1
                    issue_call(need // CH)
                    issue_call(need // CH + 1)
                    dr = dinvrep_w(w)
                    s = spool.tile([128, nwmax * 128], BF16, tag="S")
                    io_ap = iota_f[:]
                    for part, (base, n, loff) in enumerate(
                            ((lo_b, lo_n, 0), (hi_b, hi_n, NBL))):
                        if n == 0:
                            continue
                        soff = 0 if part == 0 else lo_n * 128
                        ld_ap = ldst_s[:, loff + base:loff + base + n]
                        nc.vector.tensor_tensor(
                            out=s[:, soff:soff + n * 128].rearrange(
                                "p (b q) -> p b q", b=n),
                            in0=bass.AP(io_ap.tensor, io_ap.offset,
                                        [io_ap.ap[0], [0, n], io_ap.ap[1]]),
                            in1=bass.AP(ld_ap.tensor, ld_ap.offset,
                                        [ld_ap.ap[0], ld_ap.ap[1], [0, 128]]),
                            op=mybir.AluOpType.is_equal)
                    pw = psw.tile([F, 128], FP32, tag="pw")
                    done = 0
                    for band, (base, n) in enumerate(
                            ((lo_b, lo_n), (hi_b, hi_n))):
                        for j in range(n):
                            gb = base + j
                            k = gb // CH
                            cc = (gb % CH) * 128
                            ga = gatiles[k]
                            tp = pstp.tile([128, F], FP32, tag="tpg")
                            nc.tensor.transpose(
                                out=tp[:],
                                in_=ga[band * 64:(band + 1) * 64,
                                       cc:cc + 128],
                                identity=ident[band * 64:band * 64 + 64,
                                               band * 64:band * 64 + 64])
                            gsb = gpool.tile([128, F], BF16, tag="gsb")
                            nc.vector.tensor_copy(gsb[:], tp[:])
                            nc.tensor.matmul(
                                out=pw[:], lhsT=gsb[:],
                                rhs=s[:, done * 128:(done + 1) * 128],
                                start=(done == 0),
                                stop=(done == ntot - 1))
                            done += 1
                    # drain: t = dinv[dest] (.) pw  (sign folded into W1)
                    txb = wpool.tile([F, 128], BF16, tag="txb")
                    nc.vector.tensor_tensor(out=txb[:], in0=pw[:],
                                            in1=dr[:],
                                            op=mybir.AluOpType.mult)
                    wc = slice(w * 128, (w + 1) * 128)
                    if DBG and layer == 1:
                        tdb = wpool.tile([F, 128], FP32, tag="tdb")
                        nc.vector.tensor_tensor(out=tdb[:], in0=pw[:],
                                                in1=dr[:],
                                                op=mybir.AluOpType.mult)
                        nc.sync.dma_start(
                            out=bass.AP(t_dbg.tensor, t_dbg.offset + w * 128,
                                        [[SN, 64], [1, 128]]),
                            in_=tdb[:])
                    if layer == 1:
                        # h = relu(W01.T xT + W11.T t + b1)
                        pd = psd.tile([HID, 128], FP32, tag="pt")
                        nc.tensor.matmul(out=pd[:], lhsT=w01_s[:],
                                         rhs=xT[:, wc], start=True,
                                         stop=False)
                        nc.tensor.matmul(out=pd[:], lhsT=w11_s[:],
                                         rhs=txb[:], start=False, stop=True)
                        htmp = wpool.tile([HID, 128], FP32, tag="htmp")
                        nc.scalar.activation(
                            out=htmp[:], in_=pd[:],
                            func=mybir.ActivationFunctionType.Relu,
                            bias=b1_s[:], scale=1.0)
                        nc.vector.tensor_copy(hT[:, wc], htmp[:])
                        # h' = dinv (.) h -> hp_b
                        hpb = wpool.tile([F, 128], FP32, tag="pblk")
                        nc.vector.tensor_tensor(out=hpb[:], in0=htmp[:],
                                                in1=dr[:],
                                                op=mybir.AluOpType.mult)
                        nc.sync.dma_start(
                            out=bass.AP(hp_b.tensor, hp_b.offset + w * 128,
                                        [[SN, 64], [1, 128]]),
                            in_=hpb[:])
                    else:
                        # o = W02.T hT + W12.T t + b2 ; log_softmax
                        pd = psd.tile([OUT, 128], FP32, tag="pt")
                        nc.tensor.matmul(out=pd[:], lhsT=w02_s[:],
                                         rhs=hT[:, wc], start=True,
                                         stop=False)
                        nc.tensor.matmul(out=pd[:], lhsT=w12_s[:],
                                         rhs=txb[:], start=False, stop=True)
                        ob = wpool.tile([OUT, 128], FP32, tag="ob")
                        nc.vector.tensor_scalar(
                            out=ob[:], in0=pd[:], scalar1=b2_s[:],
                            scalar2=None, op0=mybir.AluOpType.add)
                        po = psd.tile([128, OUT], FP32, tag="pt")
                        nc.tensor.transpose(out=po[:], in_=ob[:],
                                            identity=ident[:OUT, :OUT])
                        osm = wpool.tile([128, OUT], FP32, tag="osm")
                        nc.vector.tensor_copy(osm[:], po[:])
                        esm = gpool.tile([128, OUT], FP32, tag="esm")
                        nc.scalar.activation(
                            out=esm[:], in_=osm[:],
                            func=mybir.ActivationFunctionType.Exp)
                        ssum = gpool.tile([128, 1], FP32, tag="ssum")
                        nc.vector.tensor_reduce(
                            out=ssum[:], in_=esm[:], axis=AX.X,
                            op=mybir.AluOpType.add)
                        lns = gpool.tile([128, 1], FP32, tag="lns")
                        nc.scalar.activation(
                            out=lns[:], in_=ssum[:],
                            func=mybir.ActivationFunctionType.Ln)
                        ou = gpool.tile([128, OUT], FP32, tag="ou")
                        nc.vector.tensor_scalar(
                            out=ou[:], in0=osm[:], scalar1=lns[:],
                            scalar2=None, op0=mybir.AluOpType.subtract)
                        nc.sync.dma_start(
                            out=bass.AP(out.tensor, out.offset + w * 128 * OUT,
                                        [[OUT, 128], [1, OUT]]),
                            in_=ou[:])

            matvec_layer(1)
            nc.gpsimd.collective_compute(
                "AllGather", mybir.AluOpType.bypass,
                ins=[hp_b], outs=[hp_full], replica_groups=rg)
            load_table(hp_full)
            matvec_layer(2)

            rm = os.environ.get("CHEB_RM", "full")
            for _ in range(replay):
                matvec_layer(1,
                             do_gather=(rm != "nogather"),
                             do_compute=(rm != "gatheronly"))
                matvec_layer(2,
                             do_gather=(rm != "nogather"),
                             do_compute=(rm != "gatheronly"))

    nc.compile()
    return nc


# ------------------------------------------------------------------- driver

_CACHE = {}


def _get_program_and_maps(x, edge_index, W0_1, W1_1, b1, W0_2, W1_2, b2):
    idxw, ldst, rp_a, rp_b, groups, NBL, NBH = _pack(np.asarray(edge_index))

    x_pad = np.zeros((NP, F), dtype=np.float32)
    x_pad[:N] = np.asarray(x, dtype=np.float32)

    key = (NBL, NBH) + tuple(v for g in groups for v in g)
    if key not in _CACHE:
        _CACHE[key] = _build(groups, NBL, NBH)
    nc = _CACHE[key]

    shared = {
        "w01": np.asarray(W0_1, np.float32),
        "w11": -np.asarray(W1_1, np.float32),   # L_hat minus sign
        "b1": np.asarray(b1, np.float32).reshape(HID, 1),
        "w02": np.asarray(W0_2, np.float32),
        "w12": -np.asarray(W1_2, np.float32),
        "b2": np.asarray(b2, np.float32).reshape(OUT, 1),
    }
    in_maps = []
    for c in range(C):
        m = dict(shared)
        m["x_st"] = np.ascontiguousarray(x_pad[c * SN:(c + 1) * SN])
        m["idxw"] = np.ascontiguousarray(idxw[c])
        m["ldst"] = np.ascontiguousarray(ldst[c])
        m["rpa"] = np.ascontiguousarray(rp_a[c])
        m["rpb"] = np.ascontiguousarray(rp_b[c])
        in_maps.append(m)
    return nc, in_maps


def kernel(x, edge_index, W0_1, W1_1, b1, W0_2, W1_2, b2, **kw):
    nc, in_maps = _get_program_and_maps(
        x, edge_index, W0_1, W1_1, b1, W0_2, W1_2, b2)

    from concourse.bass_utils import run_bass_kernel_spmd

    res = run_bass_kernel_spmd(nc, in_maps, core_ids=list(range(C)))
    outs = [res.results[c]["out"] for c in range(C)]
    full = np.concatenate(outs, axis=0)[:N]
    return full.astype(np.float32)
